# revision 5
# baseline (speedup 1.0000x reference)
"""HEALEncoder (GNN message passing) Trainium2 kernel.

Math (reference):
    v_g   = MLP_e(concat(edge_attr, x))          # [B, E, 256], silu hidden
    s     = scatter_sum(v_g -> recv nodes)       # [B, N, 256]
    out   = MLP_f(s)                             # [B, N, 256]

Key algebraic restructure: the second linear of MLP_e commutes with the
scatter-sum, so the device only materializes h = silu(x' @ w1e + b1e) per
edge, scatter-sums h into nodes, and applies w2e (+ deg*b2e) at node level:
    s_h[n]    = sum_{e->n} silu(z_e)
    v_m_sum[n]= s_h[n] @ w2e + deg[n] * b2e
This removes ~40% of edge-level matmul work and the [B,E,256] intermediate.

Sharding: 8 cores = (batch b in 2) x (node quarter q in 4). Each core owns
nodes [3072q, 3072(q+1)) and every edge pointing at them (host sorts edges
by receiver). No collectives; each core writes a disjoint output slice.

Scatter on device: edges are host-packed into 128-edge tiles; tile j only
contains edges whose (local) receiver lies in a narrow window
[W0[w][j], W0[w][j]+32) of the current 512-node PSUM window. A host-built
one-hot [128, 32] per tile turns the scatter into a tiny matmul that
accumulates into PSUM. The first tile of each 512-node window uses a
full-width [128, 512] one-hot with start=True to initialize the bank.
The schedule W0 is computed jointly across the 4 node-quarters so one SPMD
program serves all 8 cores.

Everything compute-heavy runs in bf16 (inputs host-cast); accumulation in
fp32 PSUM. All bias terms are folded in exactly (ones-feature row for b1e,
degree row for b2e, ones-row matmuls for b1f/b2f) but are only emitted when
nonzero.
"""

import os

import numpy as np
import ml_dtypes

import concourse.bacc as bacc
import concourse.bass as bass
import concourse.mybir as mybir
import concourse.tile as tile
from concourse import bass_utils

BF16 = mybir.dt.bfloat16
F32 = mybir.dt.float32
NPBF16 = ml_dtypes.bfloat16

B, E, DX, DE, CH, NREC = 2, 393216, 128, 4, 256, 12288
NCORES, QN = 8, 4
NPC = NREC // QN  # 3072 nodes per core
NW = 6  # 512-node PSUM windows per core
WIN = NPC // NW  # 512
W = 32  # narrow scatter one-hot width
SC = 6  # L1 tiles per superchunk (one silu instruction; 3 PSUM banks)
P = 128

LAST_RUN = {}  # introspection for test harness (exec_time_ns etc.)
_PROG_CACHE = {}


# ---------------------------------------------------------------- host side


def _schedule_and_pack(rnodes_q, eids_q):
    """Jointly schedule scatter tiles for the 4 node-quarters.

    rnodes_q[q]: sorted local receiver ids (0..NPC) of core-quarter q.
    eids_q[q]:  edge ids aligned with rnodes_q[q].

    Returns (tw, w0, packs):
      tw[w]      : number of 128-edge tiles in window w (multiple of SC)
      w0[w][jl]  : node offset (within window) of tile jl's one-hot window
      packs[q]   : per q, list over global tiles of (nodes, eids) arrays
    """
    seg_nodes = {}
    seg_eids = {}
    for q in range(QN):
        r = rnodes_q[q]
        for w in range(NW):
            lo = np.searchsorted(r, w * WIN)
            hi = np.searchsorted(r, (w + 1) * WIN)
            seg_nodes[(w, q)] = r[lo:hi] - w * WIN
            seg_eids[(w, q)] = eids_q[q][lo:hi]

    tw, w0 = [], []
    packs = [[] for _ in range(QN)]
    for w in range(NW):
        ptr = [0] * QN
        offs = []
        jl = 0
        while True:
            rem = [len(seg_nodes[(w, q)]) - ptr[q] for q in range(QN)]
            if max(rem) == 0 and jl % SC == 0 and jl > 0:
                break
            if jl == 0:
                lo_b, hi_b = 0, WIN  # full-width first tile
                offs.append(0)
            else:
                cands = [
                    seg_nodes[(w, q)][ptr[q]] for q in range(QN) if rem[q] > 0
                ]
                base = int(min(cands)) if cands else WIN - W
                base = min(base, WIN - W)
                lo_b, hi_b = base, base + W
                offs.append(base)
            for q in range(QN):
                s = seg_nodes[(w, q)]
                k = ptr[q]
                kend = int(np.searchsorted(s, hi_b, side="left"))
                take = min(128, kend - k)
                packs[q].append(
                    (s[k : k + take], seg_eids[(w, q)][k : k + take])
                )
                ptr[q] = k + take
            jl += 1
        tw.append(jl)
        w0.append(offs)
    return tw, w0, packs


def _host_prep(x, edge_attr, edge_index, b1e):
    recv = np.asarray(edge_index[1]).astype(np.int64)
    order = np.argsort(recv, kind="stable")
    rs = recv[order]
    bounds = np.searchsorted(rs, np.arange(0, NREC + 1, NPC))
    rnodes_q = [
        rs[bounds[q] : bounds[q + 1]] - q * NPC for q in range(QN)
    ]
    eids_q = [order[bounds[q] : bounds[q + 1]] for q in range(QN)]

    tw, w0, packs = _schedule_and_pack(rnodes_q, eids_q)
    T = sum(tw)
    EPC = T * P

    # per-quarter, batch-independent arrays
    per_q = []
    for q in range(QN):
        cols_l, eids_l, oh_p, oh_c, ohf_p, ohf_c = [], [], [], [], [], []
        j = 0
        for w in range(NW):
            for jl in range(tw[w]):
                nodes, eids = packs[q][j]
                take = len(nodes)
                if take:
                    pp = np.arange(take)
                    cols_l.append(j * P + pp)
                    eids_l.append(eids)
                    if jl == 0:
                        ohf_p.append(pp)
                        ohf_c.append(w * WIN + nodes)
                    else:
                        oh_p.append(pp)
                        oh_c.append(j * W + (nodes - w0[w][jl]))
                j += 1
        cols = np.concatenate(cols_l)
        eids = np.concatenate(eids_l)

        oh = np.zeros((P, T * W), NPBF16)
        if oh_p:
            oh[np.concatenate(oh_p), np.concatenate(oh_c)] = 1
        ohf = np.zeros((P, NW * WIN), NPBF16)
        if ohf_p:
            ohf[np.concatenate(ohf_p), np.concatenate(ohf_c)] = 1

        ea = np.zeros((EPC, 8), np.float32)
        ea[cols, :DE] = np.asarray(edge_attr)[eids]
        ea[cols, DE] = 1.0  # ones feature -> b1e row of wea
        eat = np.ascontiguousarray(ea.T[: DE + 1]).astype(NPBF16)

        deg = np.zeros((1, NPC), np.float32)
        nl = rnodes_q[q]
        np.add.at(deg[0], nl, 1.0)

        per_q.append(
            dict(cols=cols, eids=eids, oh=oh, ohf=ohf, eat=eat,
                 deg=deg.astype(NPBF16))
        )

    # per-core x (transposed, bf16)
    xts = {}
    xf = np.asarray(x)
    for b in range(B):
        for q in range(QN):
            d = per_q[q]
            arr = np.zeros((EPC, DX), np.float32)
            arr[d["cols"]] = xf[b][d["eids"]]
            xts[(b, q)] = np.ascontiguousarray(arr.T).astype(NPBF16)

    return tw, w0, T, EPC, per_q, xts


# -------------------------------------------------------------- device side


def _build_program(tw, w0, use_b2e, use_b1f, use_b2f):
    T = sum(tw)
    EPC = T * P
    AFT = mybir.ActivationFunctionType

    nc = bacc.Bacc(
        "TRN2", target_bir_lowering=False, debug=False, num_devices=NCORES
    )
    d_xt = nc.dram_tensor("xt", [P, EPC], BF16, kind="ExternalInput")
    d_eat = nc.dram_tensor("eat", [DE + 1, EPC], BF16, kind="ExternalInput")
    d_oh = nc.dram_tensor("oh", [P, T * W], BF16, kind="ExternalInput")
    d_ohf = nc.dram_tensor("ohf", [P, NW * WIN], BF16, kind="ExternalInput")
    d_wx = nc.dram_tensor("wx", [P, CH], BF16, kind="ExternalInput")
    d_wea = nc.dram_tensor("wea", [DE + 1, CH], BF16, kind="ExternalInput")
    d_w2e = nc.dram_tensor("w2e", [P, 2 * CH], BF16, kind="ExternalInput")
    d_w1f = nc.dram_tensor("w1f", [P, 2 * CH], BF16, kind="ExternalInput")
    d_w2f = nc.dram_tensor("w2f", [P, 2 * CH], BF16, kind="ExternalInput")
    d_br = nc.dram_tensor("biasr", [1, 3 * CH], BF16, kind="ExternalInput")
    d_deg = nc.dram_tensor("deg", [1, NPC], BF16, kind="ExternalInput")
    d_out = nc.dram_tensor("out", [CH, NPC], F32, kind="ExternalOutput")

    with tile.TileContext(nc) as tc:
        with (
            tc.tile_pool(name="const", bufs=1) as cp,
            tc.tile_pool(name="stream", bufs=2) as sp,
            tc.tile_pool(name="hbuf", bufs=3) as hp,
            tc.tile_pool(name="node", bufs=2) as npool,
            tc.tile_pool(name="zpsum", bufs=2, space="PSUM") as zp,
            tc.tile_pool(name="spsum", bufs=1, space="PSUM") as ap,
        ):
            wx_t = cp.tile([P, CH], BF16)
            nc.sync.dma_start(wx_t[:], d_wx[:])
            wea_t = cp.tile([DE + 1, CH], BF16)
            nc.sync.dma_start(wea_t[:], d_wea[:])
            w2e_t = cp.tile([P, 2 * CH], BF16)
            nc.sync.dma_start(w2e_t[:], d_w2e[:])
            w1f_t = cp.tile([P, 2 * CH], BF16)
            nc.sync.dma_start(w1f_t[:], d_w1f[:])
            w2f_t = cp.tile([P, 2 * CH], BF16)
            nc.sync.dma_start(w2f_t[:], d_w2f[:])
            br_t = cp.tile([1, 3 * CH], BF16)
            nc.sync.dma_start(br_t[:], d_br[:])
            deg_t = cp.tile([1, NPC], BF16)
            nc.sync.dma_start(deg_t[:], d_deg[:])
            ohf_t = cp.tile([P, NW * WIN], BF16)
            nc.sync.dma_start(ohf_t[:], d_ohf[:])
            ones_t = cp.tile([1, WIN], BF16)
            nc.gpsimd.memset(ones_t[:], 1.0)

            jg = 0
            for w in range(NW):
                twl = tw[w]
                sacc = [
                    ap.tile([P, WIN], F32, tag=f"sacc{m}", name=f"sacc{m}")
                    for m in range(2)
                ]
                oh_w = sp.tile([P, twl * W], BF16, tag="ohw")
                nc.sync.dma_start(oh_w[:], d_oh[:, jg * W : (jg + twl) * W])
                eat_w = sp.tile([DE + 1, twl * P], BF16, tag="eatw")
                nc.sync.dma_start(
                    eat_w[:], d_eat[:, jg * P : (jg + twl) * P]
                )
                for s in range(twl // SC):
                    c0 = jg + s * SC
                    xt_t = sp.tile([P, SC * P], BF16, tag="xt")
                    nc.sync.dma_start(
                        xt_t[:], d_xt[:, c0 * P : (c0 + SC) * P]
                    )
                    z = zp.tile([P, SC * CH], F32, tag="z")
                    h = hp.tile([P, SC * CH], BF16, tag="h")
                    for t in range(SC):
                        jl = s * SC + t
                        zt = z[:, t * CH : (t + 1) * CH]
                        nc.tensor.matmul(
                            zt,
                            xt_t[:, t * P : (t + 1) * P],
                            wx_t[:],
                            start=True,
                            stop=False,
                        )
                        nc.tensor.matmul(
                            zt,
                            eat_w[:, jl * P : (jl + 1) * P],
                            wea_t[:],
                            start=False,
                            stop=True,
                        )
                    nc.scalar.activation(h[:], z[:], AFT.Silu)
                    for t in range(SC):
                        jl = s * SC + t
                        last = jl == twl - 1
                        for m in range(2):
                            lhs = h[:, t * CH + m * P : t * CH + (m + 1) * P]
                            if jl == 0:
                                nc.tensor.matmul(
                                    sacc[m][:],
                                    lhs,
                                    ohf_t[:, w * WIN : (w + 1) * WIN],
                                    start=True,
                                    stop=last,
                                )
                            else:
                                o = w0[w][jl]
                                nc.tensor.matmul(
                                    sacc[m][:, o : o + W],
                                    lhs,
                                    oh_w[:, jl * W : (jl + 1) * W],
                                    start=False,
                                    stop=last,
                                )
                jg += twl

                # ---- node phase for window w (all channel-major [*, n])
                win_sl = slice(w * WIN, (w + 1) * WIN)
                s_sb = []
                for m in range(2):
                    t_ = npool.tile([P, WIN], BF16, tag=f"s{m}", name=f"s{m}")
                    nc.vector.tensor_copy(t_[:], sacc[m][:])
                    s_sb.append(t_)

                def _layer(rhs2, wt, bslice, rhs_b, use_b, act, outdt, tag):
                    outs = []
                    for m in range(2):
                        acc = zp.tile([P, WIN], F32, tag="z", name="nacc")
                        msl0 = slice(0 * CH + m * P, 0 * CH + (m + 1) * P)
                        msl1 = slice(1 * CH + m * P, 1 * CH + (m + 1) * P)
                        nc.tensor.matmul(
                            acc[:], wt[:, msl0], rhs2[0][:],
                            start=True, stop=False,
                        )
                        nc.tensor.matmul(
                            acc[:], wt[:, msl1], rhs2[1][:],
                            start=False, stop=not use_b,
                        )
                        if use_b:
                            bsl = slice(bslice * CH + m * P,
                                        bslice * CH + (m + 1) * P)
                            nc.tensor.matmul(
                                acc[:], br_t[:, bsl], rhs_b,
                                start=False, stop=True,
                            )
                        o_ = npool.tile(
                            [P, WIN], outdt, tag=f"{tag}{m}", name=f"{tag}{m}"
                        )
                        if act:
                            nc.scalar.activation(o_[:], acc[:], AFT.Silu)
                        else:
                            nc.vector.tensor_copy(o_[:], acc[:])
                        outs.append(o_)
                    return outs

                u_sb = _layer(
                    s_sb, w2e_t, 0, deg_t[0:1, win_sl], use_b2e,
                    False, BF16, "u",
                )
                g_sb = _layer(
                    u_sb, w1f_t, 1, ones_t[:], use_b1f, True, BF16, "g"
                )
                o_sb = _layer(
                    g_sb, w2f_t, 2, ones_t[:], use_b2f, False, F32, "o"
                )
                for m in range(2):
                    nc.sync.dma_start(
                        d_out[m * P : (m + 1) * P, win_sl], o_sb[m][:]
                    )

    nc.compile()
    return nc


# ------------------------------------------------------------------- driver


def kernel(x, edge_attr, edge_index, w1e, b1e, w2e, b2e, w1f, b1f, w2f, b2f):
    x = np.asarray(x)
    tw, w0, T, EPC, per_q, xts = _host_prep(x, edge_attr, edge_index, b1e)

    use_b2e = bool(np.any(np.asarray(b2e) != 0))
    use_b1f = bool(np.any(np.asarray(b1f) != 0))
    use_b2f = bool(np.any(np.asarray(b2f) != 0))

    key = (tuple(tw), tuple(tuple(v) for v in w0), use_b2e, use_b1f, use_b2f)
    if key not in _PROG_CACHE:
        _PROG_CACHE[key] = _build_program(tw, w0, use_b2e, use_b1f, use_b2f)
    nc = _PROG_CACHE[key]

    w1e = np.asarray(w1e)
    wx = np.ascontiguousarray(w1e[DE:]).astype(NPBF16)
    wea = np.concatenate(
        [np.asarray(w1e[:DE]), np.asarray(b1e)[None]], axis=0
    ).astype(NPBF16)

    def _khalves(wm):
        wm = np.asarray(wm)
        return np.concatenate([wm[:P], wm[P:]], axis=1).astype(NPBF16)

    w2e_s = _khalves(w2e)
    w1f_s = _khalves(w1f)
    w2f_s = _khalves(w2f)
    biasr = np.concatenate(
        [np.asarray(b2e), np.asarray(b1f), np.asarray(b2f)]
    )[None].astype(NPBF16)

    in_maps = []
    for c in range(NCORES):
        b, q = divmod(c, QN)
        d = per_q[q]
        in_maps.append(
            {
                "xt": xts[(b, q)],
                "eat": d["eat"],
                "oh": d["oh"],
                "ohf": d["ohf"],
                "wx": wx,
                "wea": wea,
                "w2e": w2e_s,
                "w1f": w1f_s,
                "w2f": w2f_s,
                "biasr": biasr,
                "deg": d["deg"],
            }
        )

    trace = os.environ.get("KERNEL_TRACE", "0") == "1"
    res = bass_utils.run_bass_kernel_spmd(
        nc, in_maps, core_ids=list(range(NCORES)), trace=trace
    )
    LAST_RUN["exec_time_ns"] = res.exec_time_ns
    LAST_RUN["mean_exec_time_ns"] = res.mean_exec_time_ns
    LAST_RUN["trace"] = (
        res.instructions_and_trace[1] if res.instructions_and_trace else None
    )

    out = np.empty((B, NREC, CH), np.float32)
    for c in range(NCORES):
        b, q = divmod(c, QN)
        out[b, q * NPC : (q + 1) * NPC] = res.results[c]["out"].T
    return out
